# revision 8
# baseline (speedup 1.0000x reference)
"""Trainium2 Bass kernel for nn_FAM (dynamic grouped 3x3 low-pass filter + frequency gating).

Data-parallel over batch: 16 images -> 8 cores x 2 images.

v3: bf16-resident design.
  - Input is cast f32->bf16 during the (SWDGE) load DMA; all 32 segments of
    both images stay resident in SBUF (no ring recycling).
  - Per segment [128 h, 16 ch, 130 wpad]:
      edges  : reflect cols copied in-place (GpSimd)
      rowsum : tensor_reduce -> rsum f32 (DVE/GpSimd split)
      xs1    : one fused tensor_tensor  xs1 = seg * (s1/s2)[c]  (bf16, DVE)
  - filt branch (pool -> 1x1 conv -> BN -> tanh) identical to v2 (PE+ACT, tiny).
  - conv per segment on PE: per q-pair one [128,1024] PSUM tile accumulates
      sum_dx G_(g,dx)^T @ xs1(dx-shift)   (bf16)
    + I^T @ seg                           (bf16 identity => + x)
    + ones_col @ (beta/s2)[c]-bcast row   (f32r K=1 outer product => + beta/s2)
    so PSUM = (s1/s2)*low + x + beta/s2.
  - evacuation: out = s2[c] * PSUM, either one fused DVE tensor_tensor per
    q-pair (bcast s2) or per-channel ACT scalar.mul -- statically split per
    segment to balance DVE vs ACT load.
  - Engine budget per image ~70 us each across PE/DVE/ACT/Pool; cross-image
    interleaving keeps all engines fed while DMA streams at its descriptor-
    bound rate (~275 GB/s: 512B runs forced by the [h,c,w] gather layout).

Math: s1 = (ia+1)(ll+1)-(lh+1), s2 = lh+1, beta = -ia*(ll+1)*mean(x[c]).
out = s1*low + s2*x + beta  ==  s2 * PSUM.
"""

import os
import sys

for _p in ("/opt/trn_rl_repo", "/opt/pypackages"):
    if _p not in sys.path and os.path.isdir(_p):
        sys.path.append(_p)

from contextlib import ExitStack

import numpy as np

import concourse.bass as bass
import concourse.tile as tile
from concourse import bacc, mybir
from concourse.bass_utils import run_bass_kernel_spmd

F32 = mybir.dt.float32
F32R = mybir.dt.float32r
BF16 = mybir.dt.bfloat16
AF = mybir.ActivationFunctionType
ALU = mybir.AluOpType

N_CORES = 8
N_PER_CORE = 2        # images per core
C = 256               # channels
G = 8                 # groups
CG = C // G           # 32 channels per group
H = W = 128
HW = H * W
K = 3
BN_EPS = 1e-5
HG_CH = 16            # channels per segment / half-group
N_HG = C // HG_CH     # 16 segments per image
WPAD = W + 2          # 130: col-padded row length per channel


def _reflect(i: int) -> int:
    if i < 0:
        return -i
    if i > H - 1:
        return 2 * (H - 1) - i
    return i


def _host_consts(conv_w, bn_gamma, bn_beta, bn_mean, bn_var, lamb_l, lamb_h, inside_all):
    """Host-side parameter prep (no x-dependent math)."""
    s_bn = bn_gamma / np.sqrt(bn_var + BN_EPS)
    bn_scale = (s_bn / HW).astype(np.float32)
    bn_bias = (bn_beta - bn_mean * s_bn).astype(np.float32)
    bnsb = np.stack([bn_scale, bn_bias], axis=1)          # [72, 2]

    s1 = (inside_all + 1.0) * (lamb_l + 1.0) - (lamb_h + 1.0)
    s2 = lamb_h + 1.0
    mb = -inside_all * (lamb_l + 1.0) / HW
    sbc = np.concatenate([s1 / s2, s2]).astype(np.float32)  # [512]
    sbc = np.broadcast_to(sbc[None, :], (128, 512)).copy()  # [128, 512]
    # beta/s2 row: PSUM carries beta/s2, evac multiplies by s2.
    mbrow = (mb / s2).astype(np.float32).reshape(1, 256).copy()  # [1, 256]

    d_up = np.zeros((128, 128), np.float32)
    d_dn = np.zeros((128, 128), np.float32)
    idn = np.eye(128, dtype=np.float32)
    for h in range(H):
        d_up[_reflect(h - 1), h] = 1.0
        d_dn[_reflect(h + 1), h] = 1.0
    dmats = np.concatenate([d_up, idn, d_dn], axis=1)     # [128, 384]

    wt = conv_w.T.astype(np.float32)                      # [256, 72]
    wtd = np.concatenate([wt[:128], wt[128:]], axis=1)    # [128, 144]

    return dict(dmats=dmats, sbc=sbc, mbrow=mbrow, wtd=wtd, bnsb=bnsb)


def _build_kernel(ctx: ExitStack, tc: "tile.TileContext",
                  x_ap: bass.AP, out_ap: bass.AP,
                  dmats_ap: bass.AP, sbc_ap: bass.AP, mbrow_ap: bass.AP,
                  wtd_ap: bass.AP, bnsb_ap: bass.AP):
    nc = tc.nc

    cpool = ctx.enter_context(tc.tile_pool(name="consts", bufs=1))
    stpool = ctx.enter_context(tc.tile_pool(name="stats", bufs=1))
    segpool = ctx.enter_context(tc.tile_pool(name="seg", bufs=32))
    xspool = ctx.enter_context(tc.tile_pool(name="xs1", bufs=6))
    opool = ctx.enter_context(tc.tile_pool(name="outst", bufs=2))
    mpsum = ctx.enter_context(tc.tile_pool(name="mpsum", bufs=3, space="PSUM"))
    spsum = ctx.enter_context(tc.tile_pool(name="spsum", bufs=2, space="PSUM"))

    # ---- constants to SBUF ----
    dmats_sb = cpool.tile([128, 384], F32)
    nc.sync.dma_start(dmats_sb[:], dmats_ap)
    sbc_sb = cpool.tile([128, 512], F32)
    nc.sync.dma_start(sbc_sb[:], sbc_ap)
    mbrow_sb = cpool.tile([1, 256], F32)
    nc.sync.dma_start(mbrow_sb[:], mbrow_ap)
    wtd_sb = cpool.tile([128, 144], F32)
    nc.sync.dma_start(wtd_sb[:], wtd_ap)
    bnsb_sb = cpool.tile([72, 2], F32)
    nc.sync.dma_start(bnsb_sb[:], bnsb_ap)
    ones_sb = cpool.tile([1, 128], F32)
    nc.vector.memset(ones_sb[:], 1.0)
    onescol = cpool.tile([128, 1], F32)
    nc.vector.memset(onescol[:], 1.0)
    onesrow_r = cpool.tile([1, 128], F32R)
    nc.vector.tensor_copy(onesrow_r[:], ones_sb[:])

    idn = dmats_sb[:, 128:256]                            # [128,128] identity f32
    idn_bf = cpool.tile([128, 128], BF16)
    nc.vector.tensor_copy(idn_bf[:], idn)

    # persistent per-image tiles
    rsum, fbs, b_n, gt, prow = {}, {}, {}, {}, {}
    for n in range(N_PER_CORE):
        rsum[n] = stpool.tile([128, 256], F32, name=f"rsum_{n}")
        fbs[n] = stpool.tile([128, 72], F32, name=f"fbs_{n}")
        b_n[n] = stpool.tile([128, 256], F32R, name=f"bn_{n}")
        gt[n] = stpool.tile([128, G * 3 * 128], BF16, name=f"gt_{n}")
        prow[n] = stpool.tile([1, 256], F32, name=f"prow_{n}")

    segs = {}   # (n, hg) -> seg tile
    xs1s = {}   # (n, hg) -> xs1 tile

    def load_seg(n, hg):
        """SWDGE cast-load one segment: f32 DRAM -> bf16 SBUF."""
        c0 = hg * HG_CH
        seg = segpool.tile([128, HG_CH, WPAD], BF16, name="seg", tag="seg")
        segs[(n, hg)] = seg
        nc.gpsimd.dma_start(
            seg[:, :, 1:129],
            x_ap[n, c0:c0 + HG_CH, :, :].transpose([1, 0, 2]))

    def prep_seg(n, hg):
        """Edge reflect cols (Pool) + rowsum (split DVE/Pool) + xs1 (DVE)."""
        c0 = hg * HG_CH
        seg = segs[(n, hg)]
        nc.gpsimd.tensor_copy(seg[:, :, 0:1], seg[:, :, 2:3])
        nc.gpsimd.tensor_copy(seg[:, :, 129:130], seg[:, :, 127:128])
        # free-dim reduce is DVE-only
        nc.vector.tensor_reduce(
            out=rsum[n][:, c0:c0 + HG_CH], in_=seg[:, :, 1:129],
            axis=mybir.AxisListType.X, op=ALU.add)
        xs1 = xspool.tile([128, HG_CH, WPAD], BF16, name="xs1", tag="xs1")
        xs1s[(n, hg)] = xs1
        xeng = nc.gpsimd if hg % 3 == 1 else nc.vector
        xeng.tensor_tensor(
            out=xs1[:], in0=seg[:],
            in1=sbc_sb[:, c0:c0 + HG_CH][:, :, None].broadcast_to(
                [128, HG_CH, WPAD]),
            op=ALU.mult)

    def filt_branch(n):
        # pooled_row[1, c] = sum_h rsum[h, c]
        prp = spsum.tile([1, 256], F32, name="prp", tag="sp")
        nc.tensor.matmul(prp[:], lhsT=onescol[:], rhs=rsum[n][:],
                         start=True, stop=True)
        nc.scalar.copy(prow[n][:], prp[:])

        # conv: fpre[j] = sum_c wT[c, j] * pooled_sum[c]
        fpre = spsum.tile([72, 1], F32, name="fpre", tag="sp")
        for b in range(2):
            pcp = spsum.tile([128, 1], F32, name="pcp", tag="sp")
            nc.tensor.transpose(pcp[:], prow[n][0:1, b * 128:(b + 1) * 128],
                                idn[0:1, 0:1])
            pcol = stpool.tile([128, 1], F32, name=f"pcol_{n}_{b}")
            nc.scalar.copy(pcol[:], pcp[:])
            nc.tensor.matmul(fpre[:], lhsT=wtd_sb[:, b * 72:(b + 1) * 72],
                             rhs=pcol[:], start=(b == 0), stop=(b == 1))
        filt_sb = stpool.tile([72, 1], F32, name=f"filt_{n}")
        nc.scalar.activation(filt_sb[:], fpre[:], AF.Tanh,
                             bias=bnsb_sb[:, 1:2], scale=bnsb_sb[:, 0:1])
        # transpose [72,1] -> [1,72], then broadcast to [128,72]
        ftp = spsum.tile([1, 72], F32, name="ftp", tag="sp")
        nc.tensor.transpose(ftp[:], filt_sb[:], idn[0:72, 0:72])
        filt_row = stpool.tile([1, 72], F32, name=f"filtrow_{n}")
        nc.scalar.copy(filt_row[:], ftp[:])
        fbp = spsum.tile([128, 72], F32, name="fbp", tag="sp")
        nc.tensor.matmul(fbp[:], lhsT=ones_sb[:], rhs=filt_row[:],
                         start=True, stop=True)
        nc.scalar.copy(fbs[n][:], fbp[:])

        # beta/s2 row -> broadcast to b_n [128, 256]
        brow = stpool.tile([1, 256], F32, name=f"brow_{n}")
        nc.vector.tensor_tensor(brow[:], prow[n][:], mbrow_sb[:], op=ALU.mult)
        for b in range(2):
            bbp = spsum.tile([128, 128], F32, name="bbp", tag="sp")
            nc.tensor.matmul(bbp[:], lhsT=ones_sb[:],
                             rhs=brow[0:1, b * 128:(b + 1) * 128],
                             start=True, stop=True)
            nc.scalar.copy(b_n[n][:, b * 128:(b + 1) * 128], bbp[:])

    gtmp = stpool.tile([128, G * 3 * 128], BF16, name="gtmp")

    def g_build(n):
        # gt[:, (g,dx), :] = sum_dy fbs[g*9+dy*3+dx] * D_dy  -- 5 fused DVE ops
        gt4 = gt[n].rearrange("p (g dx w) -> p g dx w", g=G, dx=3)
        tm4 = gtmp.rearrange("p (g dx w) -> p g dx w", g=G, dx=3)
        fb4 = fbs[n].rearrange("p (g dy dx) -> p g dy dx", g=G, dy=3)
        for dy in range(3):
            dmb = dmats_sb[:, dy * 128:(dy + 1) * 128][:, None, None, :] \
                .broadcast_to([128, G, 3, 128])
            fsb = fb4[:, :, dy, :][:, :, :, None].broadcast_to([128, G, 3, 128])
            dst4 = gt4 if dy == 0 else tm4
            nc.vector.tensor_tensor(out=dst4[:], in0=dmb, in1=fsb, op=ALU.mult)
            if dy > 0:
                nc.vector.tensor_tensor(out=gt[n][:], in0=gt[n][:],
                                        in1=gtmp[:], op=ALU.add)

    def conv_seg(n, hg):
        c0 = hg * HG_CH
        g = c0 // CG
        seg = segs.pop((n, hg))
        xs1 = xs1s.pop((n, hg))
        outst = opool.tile([128, HG_CH, W], F32, name="outst")
        evac_act = (hg % 2 == 0)    # 8 segs ACT, 8 segs DVE per image
        b_nr = b_n[n]
        for qp in range(2):
            ps = mpsum.tile([128, 1024], F32, name="ps", tag="ps")
            for dx in range(3):
                blk = gt[n][:, (g * 3 + dx) * 128:(g * 3 + dx + 1) * 128]
                for qi in range(2):
                    q = qp * 2 + qi
                    nc.tensor.matmul(
                        ps[:, qi * 512:(qi + 1) * 512], lhsT=blk,
                        rhs=xs1[:, 4 * q:4 * q + 4, dx:dx + 128],
                        start=(dx == 0), stop=False)
            for qi in range(2):
                q = qp * 2 + qi
                nc.tensor.matmul(
                    ps[:, qi * 512:(qi + 1) * 512], lhsT=idn_bf[:],
                    rhs=seg[:, 4 * q:4 * q + 4, 1:129],
                    start=False, stop=False)
            for qi in range(2):
                q = qp * 2 + qi
                nc.tensor.matmul(
                    ps[:, qi * 512:(qi + 1) * 512], lhsT=onesrow_r[:],
                    rhs=b_nr[0:1, c0 + 4 * q:c0 + 4 * q + 4][:, :, None]
                        .broadcast_to([1, 4, 128]),
                    start=False, stop=True)
            ps3 = ps.rearrange("p (c w) -> p c w", c=2 * 4)
            if evac_act:
                for ci in range(8):
                    c = c0 + qp * 8 + ci
                    nc.scalar.mul(outst[:, qp * 8 + ci, :], ps3[:, ci, :],
                                  sbc_sb[:, 256 + c:256 + c + 1])
            else:
                nc.vector.tensor_tensor(
                    out=outst[:, qp * 8:(qp + 1) * 8, :], in0=ps3[:],
                    in1=sbc_sb[:, 256 + c0 + qp * 8:256 + c0 + (qp + 1) * 8]
                        [:, :, None].broadcast_to([128, 8, W]),
                    op=ALU.mult)
        nc.sync.dma_start(out_ap[n, c0:c0 + HG_CH, :, :].transpose([1, 0, 2]),
                          outst[:, :, :])

    # ---------- schedule ----------
    for hg in range(N_HG):
        load_seg(0, hg)
    for hg in range(N_HG):
        load_seg(1, hg)       # SWDGE descriptor gen is cheap; data lands later
        prep_seg(0, hg)
    filt_branch(0)
    g_build(0)
    for hg in range(N_HG):
        conv_seg(0, hg)
        prep_seg(1, hg)
    filt_branch(1)
    g_build(1)
    for hg in range(N_HG):
        conv_seg(1, hg)


def build_nc():
    nc = bacc.Bacc("TRN2", target_bir_lowering=False, debug=False)
    x_h = nc.dram_tensor("x", [N_PER_CORE, C, H, W], F32, kind="ExternalInput")
    dmats_h = nc.dram_tensor("dmats", [128, 384], F32, kind="ExternalInput")
    sbc_h = nc.dram_tensor("sbc", [128, 512], F32, kind="ExternalInput")
    mbrow_h = nc.dram_tensor("mbrow", [1, 256], F32, kind="ExternalInput")
    wtd_h = nc.dram_tensor("wtd", [128, 144], F32, kind="ExternalInput")
    bnsb_h = nc.dram_tensor("bnsb", [72, 2], F32, kind="ExternalInput")
    out_h = nc.dram_tensor("out", [N_PER_CORE, C, H, W], F32, kind="ExternalOutput")

    with tile.TileContext(nc) as tc:
        with ExitStack() as ctx:
            _build_kernel(ctx, tc, x_h.ap(), out_h.ap(), dmats_h.ap(),
                          sbc_h.ap(), mbrow_h.ap(), wtd_h.ap(), bnsb_h.ap())
    nc.compile()
    return nc


def kernel(x, conv_w, bn_gamma, bn_beta, bn_mean, bn_var, lamb_l, lamb_h,
           inside_all, _trace=False, _trace_kwargs=None):
    x = np.ascontiguousarray(x, dtype=np.float32)
    consts = _host_consts(conv_w, bn_gamma, bn_beta, bn_mean, bn_var,
                          lamb_l, lamb_h, inside_all)
    nc = build_nc()
    in_maps = []
    for i in range(N_CORES):
        m = {"x": x[i * N_PER_CORE:(i + 1) * N_PER_CORE]}
        m.update(consts)
        in_maps.append(m)
    kw = {}
    if _trace:
        kw["trace"] = True
        if _trace_kwargs:
            kw.update(_trace_kwargs)
    res = run_bass_kernel_spmd(nc, in_maps, list(range(N_CORES)), **kw)
    out = np.concatenate([res.results[i]["out"] for i in range(N_CORES)], axis=0)
    if _trace:
        kernel.last_results = res
    return out
